# revision 6
# baseline (speedup 1.0000x reference)
"""Trainium2 Bass kernel for nn_AstraloraLayer: y = x @ A.T, A = w.reshape(512, 512).

Sharding: data-parallel over the flattened token dim. x (8, 8192, 512) -> 65536
tokens, 8192 per core; w replicated (U,S,V unused in the forward). The host
pre-transposes each x shard to [512, 8192] so the contraction dim (d_in) lands
on SBUF partitions with fully contiguous DMA, and feeds A.T [d_in, d_out] so
weight chunks load naturally. Output returns in natural [tokens, d_out] layout.

Per core: 4-deep K accumulation (512 = 4 x 128) into rotating PSUM banks,
64 token tiles of 128. Raw Bass engine programs:
  SP     - weight DMA + double-buffered 4 MiB x-block DMAs
  PE     - 4-matmul accumulation groups (x tile stationary, A.T chunk moving)
  DVE    - PSUM -> SBUF copies (4 tiles batched per output slot)
  ACT    - 1 MiB output DMAs on the second HWDGE ring
"""

import numpy as np

import concourse.bass as bass
import concourse.mybir as mybir
from concourse.bass_utils import run_bass_kernel_spmd

N_CORES = 8
D_IN = 512
D_OUT = 512
TOK = 8192  # tokens per core
KC = 128  # contraction chunk (partition dim)
NK = D_IN // KC  # 4
TBLK = 2048  # tokens per x DMA block
NBLK = TOK // TBLK
TPB = TBLK // 128  # matmul tiles per block
TT = TOK // 128  # total matmul tiles
NPS = 4  # rotating PSUM banks
OBT = 4  # tiles per output DMA
NOB = 2  # output staging slots

# "f32" -> exact fp32 matmul (4 cycles/row); "f32r" -> replicated-mode fp32
# (1 cycle/row at N=512)
COMPUTE = "f32r"


def build_kernel(compute=COMPUTE):
    # in_dt: dtype of x/w DRAM params + their SBUF tiles (storage is 4-byte
    # f32 bits either way; f32r flips the PE into replicated-fp32 mode).
    in_dt = mybir.dt.float32r if compute == "f32r" else mybir.dt.float32
    nc = bass.Bass()
    xT = nc.declare_dram_parameter("xT", [D_IN, TOK], in_dt, isOutput=False)
    aT = nc.declare_dram_parameter("aT", [D_IN, D_OUT], in_dt, isOutput=False)
    out = nc.declare_dram_parameter("out", [TOK, D_OUT], mybir.dt.float32, isOutput=True)

    def mm_ap(ap):
        return ap

    with (
        nc.sbuf_tensor([KC, NK * D_OUT], in_dt) as wsb,
        nc.sbuf_tensor([KC, NK * TBLK], in_dt) as xsb0,
        nc.sbuf_tensor([KC, NK * TBLK], in_dt) as xsb1,
        nc.sbuf_tensor([128, OBT * D_OUT], mybir.dt.float32) as ob0,
        nc.sbuf_tensor([128, OBT * D_OUT], mybir.dt.float32) as ob1,
        nc.psum_tensor([128, D_OUT], mybir.dt.float32) as ps0,
        nc.psum_tensor([128, D_OUT], mybir.dt.float32) as ps1,
        nc.psum_tensor([128, D_OUT], mybir.dt.float32) as ps2,
        nc.psum_tensor([128, D_OUT], mybir.dt.float32) as ps3,
        nc.semaphore("w_sem") as w_sem,
        nc.semaphore("x_sem") as x_sem,
        nc.semaphore("mm_sem") as mm_sem,
        nc.semaphore("cp_sem") as cp_sem,
        nc.semaphore("o_sem") as o_sem,
        nc.Block() as block,
    ):
        xsb = [xsb0, xsb1]
        obuf = [ob0, ob1]
        ps = [ps0, ps1, ps2, ps3]

        @block.sync
        def _(sync):
            sync.dma_start(
                out=wsb[:, :].rearrange("p (k o) -> p k o", k=NK),
                in_=aT[:, :].rearrange("(k p) o -> p k o", p=KC),
            ).then_inc(w_sem, 16)
            for b in range(NBLK):
                if b >= 2:
                    sync.wait_ge(mm_sem, TPB * (b - 1))
                sync.dma_start(
                    out=xsb[b % 2][:, :].rearrange("p (k t) -> p k t", k=NK),
                    in_=xT[:, b * TBLK : (b + 1) * TBLK].rearrange(
                        "(k p) t -> p k t", p=KC
                    ),
                ).then_inc(x_sem, 16)

        @block.tensor
        def _(tensor):
            tensor.wait_ge(w_sem, 16)
            for b in range(NBLK):
                tensor.wait_ge(x_sem, 16 * (b + 1))
                for t in range(TPB):
                    g = b * TPB + t
                    if g >= NPS:
                        tensor.wait_ge(cp_sem, g - NPS + 1)
                    for k in range(NK):
                        mm = tensor.matmul(
                            ps[g % NPS][:, :],
                            mm_ap(
                                xsb[b % 2][:, k * TBLK + t * 128 : k * TBLK + (t + 1) * 128]
                            ),
                            mm_ap(wsb[:, k * D_OUT : (k + 1) * D_OUT]),
                            start=(k == 0),
                            stop=(k == NK - 1),
                        )
                    mm.then_inc(mm_sem, 1)

        @block.vector
        def _(vector):
            for g in range(TT):
                j = g // OBT
                pos = g % OBT
                vector.wait_ge(mm_sem, g + 1)
                if pos == 0 and j >= NOB:
                    vector.wait_ge(o_sem, 16 * (j - NOB + 1))
                vector.tensor_copy(
                    out=obuf[j % NOB][:, pos * D_OUT : (pos + 1) * D_OUT],
                    in_=ps[g % NPS][:, :],
                ).then_inc(cp_sem, 1)

        @block.scalar
        def _(scalar):
            for j in range(TT // OBT):
                scalar.wait_ge(cp_sem, OBT * (j + 1))
                tok0 = j * OBT * 128
                scalar.dma_start(
                    out=out[tok0 : tok0 + OBT * 128, :].rearrange(
                        "(a p) o -> p a o", p=128
                    ),
                    in_=obuf[j % NOB][:, :].rearrange("p (a o) -> p a o", a=OBT),
                ).then_inc(o_sem, 16)
            scalar.wait_ge(o_sem, 16 * (TT // OBT))

    return nc


def _prep_inputs(x, w):
    xf = np.asarray(x, dtype=np.float32).reshape(-1, D_IN)
    A = np.asarray(w, dtype=np.float32).reshape(D_OUT, D_IN)
    aT = np.ascontiguousarray(A.T)
    in_maps = []
    for s in range(N_CORES):
        xs = xf[s * TOK : (s + 1) * TOK]
        in_maps.append({"xT": np.ascontiguousarray(xs.T), "aT": aT})
    return in_maps


def kernel(x, w, U=None, S=None, V=None, **_):
    nc = build_kernel()
    in_maps = _prep_inputs(x, w)
    res = run_bass_kernel_spmd(nc, in_maps, core_ids=list(range(N_CORES)))
    y = np.concatenate([res.results[i]["out"] for i in range(N_CORES)], axis=0)
    return y.reshape(*x.shape[:-1], D_OUT)


# revision 7
# speedup vs baseline: 1.3269x; 1.3269x over previous
"""Trainium2 Bass kernel for nn_AstraloraLayer: y = x @ A.T, A = w.reshape(512, 512).

Sharding: data-parallel over the flattened token dim. x (8, 8192, 512) -> 65536
tokens, 8192 per core; w replicated (U,S,V unused in the forward). The host
pre-transposes each x shard to [512, 8192] so the contraction dim (d_in) lands
on SBUF partitions with fully contiguous DMA, and feeds A.T [d_in, d_out] so
weight chunks load naturally. Output returns in natural [tokens, d_out] layout.

Per core: 4-deep K accumulation (512 = 4 x 128) into rotating PSUM banks,
64 token tiles of 128. Raw Bass engine programs:
  SP     - weight DMA + double-buffered x-block DMAs
  PE     - 4-matmul accumulation groups (x tile stationary, A.T chunk moving)
  DVE    - PSUM -> SBUF copies (batched per output slot, casts to out dtype)
  ACT    - batched output DMAs on the second HWDGE ring

COMPUTE modes: "bf16" (default; inputs/outputs bf16 on the wire, f32 PSUM
accumulate, rel err ~3e-3), "f32r" (fp32 storage, replicated-mode matmul,
rel err ~1.5e-4), "f32" (exact, 4x slower PE).
"""

import numpy as np

import concourse.bass as bass
import concourse.mybir as mybir
from concourse.bass_utils import run_bass_kernel_spmd

N_CORES = 8
D_IN = 512
D_OUT = 512
TOK = 8192  # tokens per core
KC = 128  # contraction chunk (partition dim)
NK = D_IN // KC  # 4
TBLK = 2048  # tokens per x DMA block
NBLK = TOK // TBLK
TPB = TBLK // 128  # matmul tiles per block
TT = TOK // 128  # total matmul tiles
NPS = 4  # rotating PSUM banks
OBT = 4  # tiles per output DMA
NOB = 2  # output staging slots

COMPUTE = "bf16"


def build_kernel(compute=COMPUTE):
    if compute == "bf16":
        in_dt = mybir.dt.bfloat16
        out_dt = mybir.dt.bfloat16
    elif compute == "f32r":
        in_dt = mybir.dt.float32r
        out_dt = mybir.dt.float32
    else:
        in_dt = mybir.dt.float32
        out_dt = mybir.dt.float32

    nc = bass.Bass()
    xT = nc.declare_dram_parameter("xT", [D_IN, TOK], in_dt, isOutput=False)
    aT = nc.declare_dram_parameter("aT", [D_IN, D_OUT], in_dt, isOutput=False)
    out = nc.declare_dram_parameter("out", [TOK, D_OUT], out_dt, isOutput=True)

    with (
        nc.sbuf_tensor([KC, NK * D_OUT], in_dt) as wsb,
        nc.sbuf_tensor([KC, NK * TBLK], in_dt) as xsb0,
        nc.sbuf_tensor([KC, NK * TBLK], in_dt) as xsb1,
        nc.sbuf_tensor([128, OBT * D_OUT], out_dt) as ob0,
        nc.sbuf_tensor([128, OBT * D_OUT], out_dt) as ob1,
        nc.psum_tensor([128, D_OUT], mybir.dt.float32) as ps0,
        nc.psum_tensor([128, D_OUT], mybir.dt.float32) as ps1,
        nc.psum_tensor([128, D_OUT], mybir.dt.float32) as ps2,
        nc.psum_tensor([128, D_OUT], mybir.dt.float32) as ps3,
        nc.semaphore("w_sem") as w_sem,
        nc.semaphore("x_sem") as x_sem,
        nc.semaphore("mm_sem") as mm_sem,
        nc.semaphore("cp_sem") as cp_sem,
        nc.semaphore("o_sem") as o_sem,
        nc.Block() as block,
    ):
        xsb = [xsb0, xsb1]
        obuf = [ob0, ob1]
        ps = [ps0, ps1, ps2, ps3]

        @block.sync
        def _(sync):
            sync.dma_start(
                out=wsb[:, :].rearrange("p (k o) -> p k o", k=NK),
                in_=aT[:, :].rearrange("(k p) o -> p k o", p=KC),
            ).then_inc(w_sem, 16)
            for b in range(NBLK):
                if b >= 2:
                    sync.wait_ge(mm_sem, TPB * (b - 1))
                sync.dma_start(
                    out=xsb[b % 2][:, :].rearrange("p (k t) -> p k t", k=NK),
                    in_=xT[:, b * TBLK : (b + 1) * TBLK].rearrange(
                        "(k p) t -> p k t", p=KC
                    ),
                ).then_inc(x_sem, 16)

        @block.tensor
        def _(tensor):
            tensor.wait_ge(w_sem, 16)
            for b in range(NBLK):
                tensor.wait_ge(x_sem, 16 * (b + 1))
                for t in range(TPB):
                    g = b * TPB + t
                    if g >= NPS:
                        tensor.wait_ge(cp_sem, g - NPS + 1)
                    for k in range(NK):
                        mm = tensor.matmul(
                            ps[g % NPS][:, :],
                            xsb[b % 2][
                                :, k * TBLK + t * 128 : k * TBLK + (t + 1) * 128
                            ],
                            wsb[:, k * D_OUT : (k + 1) * D_OUT],
                            start=(k == 0),
                            stop=(k == NK - 1),
                        )
                    mm.then_inc(mm_sem, 1)

        @block.vector
        def _(vector):
            for g in range(TT):
                j = g // OBT
                pos = g % OBT
                vector.wait_ge(mm_sem, g + 1)
                if pos == 0 and j >= NOB:
                    vector.wait_ge(o_sem, 16 * (j - NOB + 1))
                vector.tensor_copy(
                    out=obuf[j % NOB][:, pos * D_OUT : (pos + 1) * D_OUT],
                    in_=ps[g % NPS][:, :],
                ).then_inc(cp_sem, 1)

        @block.scalar
        def _(scalar):
            for j in range(TT // OBT):
                scalar.wait_ge(cp_sem, OBT * (j + 1))
                tok0 = j * OBT * 128
                scalar.dma_start(
                    out=out[tok0 : tok0 + OBT * 128, :].rearrange(
                        "(a p) o -> p a o", p=128
                    ),
                    in_=obuf[j % NOB][:, :].rearrange("p (a o) -> p a o", a=OBT),
                ).then_inc(o_sem, 16)
            scalar.wait_ge(o_sem, 16 * (TT // OBT))

    return nc


def _prep_inputs(x, w, compute=COMPUTE):
    if compute == "bf16":
        import ml_dtypes

        np_dt = ml_dtypes.bfloat16
    else:
        np_dt = np.float32
    xf = np.asarray(x, dtype=np.float32).reshape(-1, D_IN)
    A = np.asarray(w, dtype=np.float32).reshape(D_OUT, D_IN)
    aT = np.ascontiguousarray(A.T).astype(np_dt)
    in_maps = []
    for s in range(N_CORES):
        xs = xf[s * TOK : (s + 1) * TOK]
        in_maps.append({"xT": np.ascontiguousarray(xs.T).astype(np_dt), "aT": aT})
    return in_maps


def kernel(x, w, U=None, S=None, V=None, **_):
    nc = build_kernel()
    in_maps = _prep_inputs(x, w)
    res = run_bass_kernel_spmd(nc, in_maps, core_ids=list(range(N_CORES)))
    y = np.concatenate(
        [np.asarray(res.results[i]["out"], dtype=np.float32) for i in range(N_CORES)],
        axis=0,
    )
    return y.reshape(*x.shape[:-1], D_OUT)
